# revision 33
# baseline (speedup 1.0000x reference)
"""Trainium2 Bass kernel for MoD (mixture-of-depths) routing FFN.

Semantics (matching the reference):
  w = x @ W_r + b_r                        # [B, S] router weights
  t_b = K-th largest of w[b, :]            # per-row threshold (K=512)
  selected: w > t_b (strict; ties at threshold dropped)
  out[b, s] = w[b,s] * (gelu(x[b,s] @ W1 + b1) @ W2 + b2)   if selected
  out[b, s] = x[b, s]                                        otherwise

Sharding: 8 cores; cores (2b, 2b+1) form a pair handling batch row b.
Each core routes half the row; the pair AllGathers the 8KB router
vector; the per-row top-K threshold is found with a 3-pass 128-way
grid refinement (each pass one DVE compare-accumulate over [128, S]),
which replaces the O(S^2) rank-count of the previous version. The
selected tokens are compacted into K slots via matmul-based stream
compaction, gathered, and the FFN runs tensor-parallel over the pair
(W1 column / W2 row split, bf16) with per-512-column-group bf16
AllReduces overlapped with MM2; each group is scaled by the router
gate and scattered as soon as its AllReduce lands. The residual
(out = x) is written from SBUF during the router phase.
"""

from contextlib import ExitStack

import numpy as np

import concourse.bass as bass
import concourse.tile as tile
from concourse import mybir
from concourse.bass import IndirectOffsetOnAxis
from concourse.bass_utils import run_bass_kernel_spmd
from concourse.masks import make_identity
from concourse.tile_rust import add_dep_helper

F32 = mybir.dt.float32
BF16 = mybir.dt.bfloat16
I32 = mybir.dt.int32

NC_CORES = 8

# 3-pass threshold search grid: range [-8, 8), 128 points per pass.
TAU_LO0 = -8.0
TAU_STEP = [16.0 / 128.0, 16.0 / 128.0**2, 16.0 / 128.0**3]


def build_mod_kernel(nc, S, D, DFF, K):
    """Emit the per-core SPMD program. Pair = (2b, 2b+1) handles row b.

    Inputs (per-core):
      x_own  [S/2, D] f32   this core's half-row (router + residual src)
      x_row  [S, D]   f32   the full row (gather source for the FFN)
      wr     [1, D]   f32   router weight
      br     [1, 1]   f32   router bias
      w1     [8, 4, 128, 2048] bf16  W1 column shard, packed (mg, kk)
      w2     [4, 8, 128, 2048] bf16  W2 row shard, packed (g, mm)
      b1s    [128, NM] f32  b1 shard (pre-transposed)
      b2f    [1, D]   f32   full b2
      hoff   [1, 1]   f32   h * S/2  (0 for even cores, S/2 for odd)
    Output:
      out    [S/2, D] f32
    """
    HALF = S // 2
    DFFH = DFF // 2
    CAP = K                      # slots per row (max selected = K-1 < CAP)
    KT = HALF // 128             # x tiles per core (16)
    TT = S // 128                # token tiles per row (32)
    NJ = CAP // 128              # slot tiles (4)
    ND = D // 128                # d 128-tiles (16)
    NM = DFFH // 128             # dff-col tiles (32)
    MG = 4                       # m-tiles per MM1 psum group
    NMG = NM // MG               # MM1 groups (8)
    KK = 4                       # d-chunks packed per w1 DMA
    NKK = ND // KK               # w1 DMAs per mg (4)
    ML = 4                       # m-tiles packed per w2 DMA
    NML = NM // ML               # w2 DMAs per group (8)
    NGRP = D // 512              # MM2 col groups == AllReduce chunks (4)

    x_own = nc.declare_dram_parameter("x_own", [HALF, D], F32, isOutput=False)
    x_row = nc.declare_dram_parameter("x_row", [S, D], F32, isOutput=False)
    wr = nc.declare_dram_parameter("wr", [1, D], F32, isOutput=False)
    br = nc.declare_dram_parameter("br", [1, 1], F32, isOutput=False)
    w1 = nc.declare_dram_parameter("w1", [NMG, NKK, 128, KK * MG * 128], BF16,
                                   isOutput=False)
    w2 = nc.declare_dram_parameter("w2", [NGRP, NML, 128, ML * 512], BF16,
                                   isOutput=False)
    b1s = nc.declare_dram_parameter("b1s", [128, NM], F32, isOutput=False)
    b2h = nc.declare_dram_parameter("b2h", [1, D], BF16, isOutput=False)
    hoff = nc.declare_dram_parameter("hoff", [1, 1], F32, isOutput=False)
    out = nc.declare_dram_parameter("out", [HALF, D], F32, isOutput=True)

    # Internal DRAM for collectives (pair groups).
    ag_in = nc.dram_tensor("ag_in", [1, HALF], F32)
    ag_out = nc.dram_tensor("ag_out", [2, HALF], F32)
    ar_in = nc.dram_tensor("ar_in", [NGRP, CAP, 512], BF16)
    ar_out = nc.dram_tensor("ar_out", [NGRP, CAP, 512], BF16)
    pairs = [[2 * b, 2 * b + 1] for b in range(NC_CORES // 2)]

    with tile.TileContext(nc) as tc, ExitStack() as ctx:
        pc = ctx.enter_context(tc.tile_pool(name="const", bufs=1))
        pr = ctx.enter_context(tc.tile_pool(name="route", bufs=1))

        # ---- small input broadcasts ----
        wr1 = pc.tile([1, D], F32, name="wr1")
        nc.sync.dma_start(wr1[:], wr.ap())
        wr_bc = pc.tile([128, D], F32, name="wr_bc")
        nc.gpsimd.partition_broadcast(wr_bc[:], wr1[:], 128)
        br1 = pc.tile([1, 1], F32, name="br1")
        nc.sync.dma_start(br1[:], br.ap())
        br_bc = pc.tile([128, 1], F32, name="br_bc")
        nc.gpsimd.partition_broadcast(br_bc[:], br1[:], 128)
        ho1 = pc.tile([1, 1], F32, name="ho1")
        nc.sync.dma_start(ho1[:], hoff.ap())
        ho_bc = pc.tile([128, 1], F32, name="ho_bc")
        nc.gpsimd.partition_broadcast(ho_bc[:], ho1[:], 128)
        # b1_sb[p, m] = b1[m*128 + p] (host pre-transposed)
        b1_sb = pc.tile([128, NM], F32, name="b1_sb")
        nc.sync.dma_start(b1_sb[:], b1s.ap())
        b2_sb = pc.tile([1, D], BF16, name="b2_sb")
        nc.sync.dma_start(b2_sb[:], b2h.ap())

        # ---- constants ----
        ident = pc.tile([128, 128], F32, name="ident")
        make_identity(nc, ident[:])
        ones128 = pc.tile([128, 1], F32, name="ones128")
        nc.vector.memset(ones128[:], 1.0)
        ones1b = pc.tile([1, 128], BF16, name="ones1b")
        nc.vector.memset(ones1b[:], 1.0)
        # iota_c[p, 0] = p
        iota_c = pc.tile([128, 1], F32, name="iota_c")
        nc.gpsimd.iota(iota_c[:], pattern=[[0, 1]], base=0,
                       channel_multiplier=1,
                       allow_small_or_imprecise_dtypes=True)
        # U strict-upper triangulars (as stored): U[q, p] = 1 iff q < p
        uTT = pc.tile([TT, TT], F32, name="uTT")
        nc.gpsimd.memset(uTT[:], 0.0)
        nc.gpsimd.affine_select(
            out=uTT[:], in_=uTT[:], compare_op=mybir.AluOpType.is_ge,
            fill=1.0, base=0, pattern=[[-1, TT]], channel_multiplier=1,
        )
        u128 = pc.tile([128, 128], F32, name="u128")
        nc.gpsimd.memset(u128[:], 0.0)
        nc.gpsimd.affine_select(
            out=u128[:], in_=u128[:], compare_op=mybir.AluOpType.is_ge,
            fill=1.0, base=0, pattern=[[-1, 128]], channel_multiplier=1,
        )
        s_iota = pc.tile([128, CAP], F32, name="s_iota")
        nc.gpsimd.iota(s_iota[:], pattern=[[1, CAP]], base=0,
                       channel_multiplier=0, allow_small_or_imprecise_dtypes=True)
        # compact lhsT rows, bf16-exact: [p+1, c, gate] per token column c
        tg3 = pc.tile([128, 3 * TT], BF16, name="tg3")
        tg3v = tg3[:].rearrange("p (c three) -> p c three", three=3)
        nc.gpsimd.iota(tg3v[:, :, 0], pattern=[[0, TT]], base=1,
                       channel_multiplier=1, allow_small_or_imprecise_dtypes=True)
        nc.gpsimd.iota(tg3v[:, :, 1], pattern=[[1, TT]], base=0,
                       channel_multiplier=0, allow_small_or_imprecise_dtypes=True)

        # ---- phase R: router dot ----
        w_mine = pr.tile([128, KT], F32, name="w_mine")
        with tc.tile_pool(name="xs", bufs=5) as px, \
             tc.tile_pool(name="jr", bufs=1) as pjr:
            for k in range(KT):
                xt = px.tile([128, D], F32)
                eng = nc.sync if k % 2 == 0 else nc.scalar
                eng.dma_start(xt[:], x_own.ap()[k * 128:(k + 1) * 128, :])
                jt = pjr.tile([128, D], F32, tag="jR")
                nc.vector.scalar_tensor_tensor(
                    out=jt[:], in0=xt[:], scalar=1.0, in1=wr_bc[:],
                    op0=mybir.AluOpType.bypass, op1=mybir.AluOpType.mult,
                    accum_out=w_mine[:, k:k + 1],
                )
            w_full = pr.tile([128, KT], F32, name="w_full")
            nc.vector.tensor_scalar_add(w_full[:], w_mine[:], br_bc[:, 0:1])
            # transpose to [KT, 128] so the DRAM write (l = k*128 + p) is
            # contiguous instead of a 4-byte-packet strided DMA
            with tc.tile_pool(name="pwt", bufs=1, space="PSUM") as pwt:
                wfT_ps = pwt.tile([KT, 128], F32, name="wfT_ps")
                nc.tensor.transpose(wfT_ps[:], w_full[:], ident[:])
                wfT = pr.tile([KT, 128], F32, name="wfT")
                nc.vector.tensor_copy(wfT[:], wfT_ps[:])
            nc.sync.dma_start(
                ag_in.ap().rearrange("o (k p) -> (o k) p", p=128), wfT[:])

        # ---- AllGather router weights within pair ----
        nc.gpsimd.collective_compute(
            "AllGather", mybir.AluOpType.bypass, replica_groups=pairs,
            ins=[ag_in.ap()], outs=[ag_out.ap()],
        )

        # ---- load full row back: replicated [128, S] + token-major ----
        wrow = pr.tile([1, S], F32, name="wrow")
        nc.sync.dma_start(wrow[:, 0:HALF], ag_out.ap()[0:1, :])
        nc.sync.dma_start(wrow[:, HALF:S], ag_out.ap()[1:2, :])
        w_bc = pr.tile([128, S], F32, name="w_bc")
        nc.gpsimd.partition_broadcast(w_bc[:], wrow[:], 128)
        # w32[c, p] = w[c*128 + p]  (contiguous), one transpose -> w_tok
        w32 = pr.tile([TT, 128], F32, name="w32")
        nc.sync.dma_start(
            w32[:], ag_out.ap().rearrange("o (c p) -> (o c) p", p=128))
        w_tok = pr.tile([128, TT], F32, name="w_tok")
        with tc.tile_pool(name="pwk", bufs=1, space="PSUM") as pwk:
            wt_ps = pwk.tile([128, TT], F32, name="wt_ps")
            nc.tensor.transpose(wt_ps[:], w32[:], ident[0:TT, 0:TT])
            nc.vector.tensor_copy(w_tok[:], wt_ps[:])

        # ---- phase TAU: 3-pass 128-way threshold grid refinement ----
        # Invariant per pass: tau* = w_(K) lies in (lo, lo + 128*step].
        # Final tau' = lo + s*step satisfies count(w > tau') == K-1 as long
        # as the last step < gap between w_(K-1) and w_(K).
        junk = pr.tile([128, S], BF16, name="junk")
        tau = pr.tile([128, 1], F32, name="tau")
        lo = pr.tile([1, 1], F32, name="lo")
        s_sb = pr.tile([1, 1], F32, name="s_sb")
        with tc.tile_pool(name="pts", bufs=1, space="PSUM") as pts:
            for p in range(3):
                if p == 0:
                    # static grid: tau_q = -8 + q * step0  (exact fp32)
                    nc.vector.tensor_scalar(
                        out=tau[:], in0=iota_c[:], scalar1=TAU_STEP[0],
                        scalar2=TAU_LO0, op0=mybir.AluOpType.mult,
                        op1=mybir.AluOpType.add)
                else:
                    # tau_q = fl(q*step) + lo  (bit-identical to lo update)
                    lo_bc = pr.tile([128, 1], F32, name=f"lo_bc{p}")
                    nc.gpsimd.partition_broadcast(lo_bc[:], lo[:], 128)
                    nc.vector.scalar_tensor_tensor(
                        out=tau[:], in0=iota_c[:], scalar=TAU_STEP[p],
                        in1=lo_bc[:], op0=mybir.AluOpType.mult,
                        op1=mybir.AluOpType.add)
                c_col = pr.tile([128, 1], F32, name=f"c_col{p}")
                nc.vector.tensor_scalar(
                    out=junk[:], in0=w_bc[:], scalar1=tau[:, 0:1],
                    scalar2=None, op0=mybir.AluOpType.is_gt,
                    op1=mybir.AluOpType.add, accum_out=c_col[:],
                )
                m_col = pr.tile([128, 1], F32, name=f"m_col{p}")
                nc.vector.tensor_scalar(
                    out=m_col[:], in0=c_col[:], scalar1=float(K),
                    scalar2=None, op0=mybir.AluOpType.is_ge)
                s_ps = pts.tile([1, 1], F32, tag="s_ps")
                nc.tensor.matmul(s_ps[:], lhsT=m_col[:], rhs=ones128[:],
                                 start=True, stop=True)
                nc.vector.tensor_copy(s_sb[:], s_ps[:])
                # lo_next = fl((s-1)*step) + lo   (same fp ops as the grid)
                sm1 = pr.tile([1, 1], F32, name=f"sm1_{p}")
                nc.vector.tensor_scalar(
                    out=sm1[:], in0=s_sb[:],
                    scalar1=-1.0 if p < 2 else 0.0, scalar2=TAU_STEP[p],
                    op0=mybir.AluOpType.add, op1=mybir.AluOpType.mult)
                if p == 0:
                    nc.vector.tensor_scalar_add(lo[:], sm1[:], TAU_LO0)
                else:
                    lo2 = pr.tile([1, 1], F32, name=f"lo2_{p}")
                    nc.vector.tensor_tensor(
                        out=lo2[:], in0=sm1[:], in1=lo[:],
                        op=mybir.AluOpType.add)
                    if p < 2:
                        nc.vector.tensor_copy(lo[:], lo2[:])
                    else:
                        tau_fin = lo2   # lo2 = lo + s*step3 (sm1 had s, not s-1)
        tau_bc = pr.tile([128, 1], F32, name="tau_bc")
        nc.gpsimd.partition_broadcast(tau_bc[:], tau_fin[:], 128)

        # ---- selection / gate / prefix ----
        sel = pr.tile([128, TT], F32, name="sel")
        nc.vector.tensor_scalar(out=sel[:], in0=w_tok[:],
                                scalar1=tau_bc[:, 0:1], scalar2=None,
                                op0=mybir.AluOpType.is_gt)
        unsel = pr.tile([128, TT], F32, name="unsel")
        nc.vector.tensor_scalar(out=unsel[:], in0=w_tok[:],
                                scalar1=tau_bc[:, 0:1], scalar2=None,
                                op0=mybir.AluOpType.is_le)
        gate = pr.tile([128, TT], F32, name="gate")
        nc.vector.tensor_tensor(out=gate[:], in0=sel[:], in1=w_tok[:],
                                op=mybir.AluOpType.mult)
        nc.vector.tensor_copy(tg3v[:, :, 2], gate[:])

        # exclusive prefix-sum of sel over t = c*128 + p
        with tc.tile_pool(name="pps", bufs=1, space="PSUM") as pps:
            colT_ps = pps.tile([TT, 1], F32, name="colT_ps")
            nc.tensor.matmul(colT_ps[:], lhsT=sel[:], rhs=ones128[:],
                             start=True, stop=True)
            colT = pr.tile([TT, 1], F32, name="colT")
            nc.vector.tensor_copy(colT[:], colT_ps[:])
            pos_ps = pps.tile([128, TT], F32, name="pos_ps")
            nc.tensor.matmul(pos_ps[:], lhsT=colT[:].to_broadcast([TT, 128]),
                             rhs=uTT[:], start=True, stop=False)
            nc.tensor.matmul(pos_ps[:], lhsT=u128[:], rhs=sel[:],
                             start=False, stop=True)
            pos = pr.tile([128, TT], F32, name="pos")
            nc.vector.tensor_copy(pos[:], pos_ps[:])
        pos_m = pr.tile([128, TT], F32, name="pos_m")
        nc.vector.scalar_tensor_tensor(
            out=pos_m[:], in0=unsel[:], scalar=float(4 * CAP + 7), in1=pos[:],
            op0=mybir.AluOpType.mult, op1=mybir.AluOpType.add,
        )

        # ---- phase COMPACT: slot -> (p+1, c, gate) via bf16 matmuls ----
        tok_i = []   # int32 gather offsets per slot tile
        gate_s = []  # f32 per-slot gates
        dest_i = []  # int32 scatter offsets (OOB for pad/other-half)
        with tc.tile_pool(name="pcm", bufs=1, space="PSUM") as pcm, \
             tc.tile_pool(name="pmm", bufs=4) as pmm, \
             tc.tile_pool(name="ptp", bufs=4, space="PSUM") as ptp:
            cps = pcm.tile([3, CAP], F32, name="cps")
            for c in range(TT):
                mt = pmm.tile([128, CAP], BF16, tag="mt")
                nc.vector.tensor_scalar(
                    out=mt[:], in0=s_iota[:], scalar1=pos_m[:, c:c + 1],
                    scalar2=None, op0=mybir.AluOpType.is_equal,
                )
                nc.tensor.matmul(cps[:], lhsT=tg3[:, 3 * c:3 * c + 3], rhs=mt[:],
                                 start=(c == 0), stop=(c == TT - 1))
            compact = pr.tile([3, CAP], F32, name="compact")
            nc.vector.tensor_copy(compact[:], cps[:])
            for j in range(NJ):
                tp = ptp.tile([128, 3], F32, tag="tp")
                nc.tensor.transpose(tp[:], compact[:, j * 128:(j + 1) * 128],
                                    ident[0:3, 0:3])
                cpj = pr.tile([128, 3], F32, name=f"cpj{j}")
                nc.vector.tensor_copy(cpj[:], tp[:])
                gate_s.append(cpj)
                # tokp1 = 128*c + (p+1)  == token id + 1; 0 for pad slots
                tokp1 = pr.tile([128, 1], F32, name=f"tokp1{j}")
                nc.vector.scalar_tensor_tensor(
                    out=tokp1[:], in0=cpj[:, 1:2], scalar=128.0, in1=cpj[:, 0:1],
                    op0=mybir.AluOpType.mult, op1=mybir.AluOpType.add)
                # gather offset: max(tokp1 - 1, 0) -> int
                tif = pr.tile([128, 1], F32, name=f"tif{j}")
                nc.vector.tensor_scalar(
                    out=tif[:], in0=tokp1[:], scalar1=-1.0, scalar2=0.0,
                    op0=mybir.AluOpType.add, op1=mybir.AluOpType.max,
                )
                tii = pr.tile([128, 1], I32, name=f"tii{j}")
                nc.vector.tensor_copy(tii[:], tif[:])
                tok_i.append(tii)
                # scatter offset: (tokp1 - 1) - hoff, OOB for pad/other-half
                df = pr.tile([128, 1], F32, name=f"df{j}")
                nc.vector.scalar_tensor_tensor(
                    out=df[:], in0=tokp1[:], scalar=-1.0, in1=ho_bc[:],
                    op0=mybir.AluOpType.add, op1=mybir.AluOpType.subtract,
                )
                ok1 = pr.tile([128, 1], F32, name=f"ok1{j}")
                nc.vector.tensor_scalar(out=ok1[:], in0=df[:], scalar1=0.0,
                                        scalar2=None, op0=mybir.AluOpType.is_ge)
                ok2 = pr.tile([128, 1], F32, name=f"ok2{j}")
                nc.vector.tensor_scalar(out=ok2[:], in0=df[:],
                                        scalar1=float(HALF - 1), scalar2=None,
                                        op0=mybir.AluOpType.is_le)
                okm = pr.tile([128, 1], F32, name=f"okm{j}")
                nc.vector.tensor_tensor(out=okm[:], in0=ok1[:], in1=ok2[:],
                                        op=mybir.AluOpType.mult)
                # dfm = okm * (df - BIG) + BIG  (df when ok, BIG when not)
                BIG = float(8 * HALF + 11)
                dfs = pr.tile([128, 1], F32, name=f"dfs{j}")
                nc.vector.tensor_scalar_add(dfs[:], df[:], -BIG)
                dfm = pr.tile([128, 1], F32, name=f"dfm{j}")
                nc.vector.scalar_tensor_tensor(
                    out=dfm[:], in0=okm[:], scalar=BIG, in1=dfs[:],
                    op0=mybir.AluOpType.bypass, op1=mybir.AluOpType.mult)
                nc.vector.tensor_scalar_add(dfm[:], dfm[:], BIG)
                dii = pr.tile([128, 1], I32, name=f"dii{j}")
                nc.vector.tensor_copy(dii[:], dfm[:])
                dest_i.append(dii)

        # ---- phase GATHER: xg rows -> transpose -> xgT (bf16) ----
        xgT = pr.tile([128, ND, CAP], BF16, name="xgT")
        residual_dmas = []
        with tc.tile_pool(name="pxg", bufs=2) as pxg, \
             tc.tile_pool(name="ptg", bufs=4, space="PSUM") as ptg:
            for j in range(NJ):
                xg = pxg.tile([128, D], F32, tag="xg")
                nc.gpsimd.indirect_dma_start(
                    out=xg[:], out_offset=None, in_=x_row.ap(),
                    in_offset=IndirectOffsetOnAxis(ap=tok_i[j][:, 0:1], axis=0),
                )
                # residual copy out = x (DRAM->DRAM), queued behind the
                # gather so it streams during MM1 when HBM is underused;
                # the per-group scatter overwrites selected rows later.
                r = nc.gpsimd.dma_start(
                    out.ap()[j * (HALF // NJ):(j + 1) * (HALF // NJ), :],
                    x_own.ap()[j * (HALF // NJ):(j + 1) * (HALF // NJ), :])
                residual_dmas.append(r)
                for k in range(ND):
                    tps = ptg.tile([128, 128], F32, tag="tps")
                    nc.tensor.transpose(tps[:], xg[:, k * 128:(k + 1) * 128],
                                        ident[:])
                    eng = nc.vector if k % 2 == 0 else nc.scalar
                    if k % 2 == 0:
                        nc.vector.tensor_copy(
                            xgT[:, k, j * 128:(j + 1) * 128], tps[:])
                    else:
                        nc.scalar.copy(
                            xgT[:, k, j * 128:(j + 1) * 128], tps[:])

        # ---- phase MM1 + gelu: h[dffcol, toks] = gelu(xg @ W1 + b1) ----
        # w1 DMAs ride the sync queue (scalar holds the gelus; a gelu that
        # waits for a psum stop must not block next group's weight loads)
        h_all = pr.tile([128, NM, CAP], BF16, name="h_all")
        with tc.tile_pool(name="pw1", bufs=5) as pw1, \
             tc.tile_pool(name="ph1", bufs=2, space="PSUM") as ph1:
            for mg in range(NMG):
                hps = [ph1.tile([128, CAP], F32, tag=f"hp{i}", name=f"hp{i}")
                       for i in range(MG)]
                for kk in range(NKK):
                    w1c = pw1.tile([128, KK * MG * 128], BF16, tag="w1c")
                    nc.sync.dma_start(w1c[:], w1.ap()[mg, kk])
                    for kl in range(KK):
                        k = KK * kk + kl
                        for i in range(MG):
                            nc.tensor.matmul(
                                hps[i][:],
                                lhsT=w1c[:, kl * 512 + i * 128:
                                         kl * 512 + (i + 1) * 128],
                                rhs=xgT[:, k, :],
                                start=(kk == 0 and kl == 0),
                                stop=(kk == NKK - 1 and kl == KK - 1))
                for i in range(MG):
                    m = mg * MG + i
                    nc.scalar.activation(
                        out=h_all[:, m, :], in_=hps[i][:],
                        func=mybir.ActivationFunctionType.Gelu_apprx_tanh,
                        bias=b1_sb[:, m:m + 1], scale=1.0)

        # ---- phase MM2 + chunked bf16 AllReduce + per-group scatter ----
        # w2 loads ride the gpsimd queue: the scalar queue is blocked by
        # gelus until MM1 ends, which would delay the first w2 tiles and
        # stall the MM1->MM2 transition
        with tc.tile_pool(name="pw2", bufs=5) as pw2, \
             tc.tile_pool(name="pb2", bufs=2, space="PSUM") as pb2, \
             tc.tile_pool(name="pbs", bufs=6) as pbs, \
             tc.tile_pool(name="pfa", bufs=6) as pfa:
            for g in range(NGRP):
                bps = [pb2.tile([128, 512], F32, tag=f"bp{j}", name=f"bp{j}")
                       for j in range(NJ)]
                for mm in range(NML):
                    w2c = pw2.tile([128, ML * 512], BF16, tag="w2c")
                    nc.gpsimd.dma_start(w2c[:], w2.ap()[g, mm])
                    for ml in range(ML):
                        m = ML * mm + ml
                        for j in range(NJ):
                            nc.tensor.matmul(
                                bps[j][:],
                                lhsT=h_all[:, m, j * 128:(j + 1) * 128],
                                rhs=w2c[:, ml * 512:(ml + 1) * 512],
                                start=(mm == 0 and ml == 0), stop=False)
                for j in range(NJ):
                    # bias row (0.5*b2 per core; AR doubles it back)
                    nc.tensor.matmul(
                        bps[j][:], lhsT=ones1b[:],
                        rhs=b2_sb[:, g * 512:(g + 1) * 512],
                        start=False, stop=True)
                    # pre-scale by the gate before the AllReduce:
                    # (p0 + b2/2)*gate + (p1 + b2/2)*gate = (p0+p1+b2)*gate,
                    # so post-AR work is just read + cast + scatter
                    bsb = pbs.tile([128, 512], BF16, tag="bsb")
                    nc.vector.tensor_scalar(
                        out=bsb[:], in0=bps[j][:], scalar1=gate_s[j][:, 2:3],
                        scalar2=None, op0=mybir.AluOpType.mult)
                    nc.gpsimd.dma_start(
                        ar_in.ap()[g, j * 128:(j + 1) * 128, :], bsb[:])
                nc.gpsimd.collective_compute(
                    "AllReduce", mybir.AluOpType.add, replica_groups=pairs,
                    ins=[ar_in.ap()[g]], outs=[ar_out.ap()[g]],
                )
                # final combine of group g: read, widen to f32, scatter
                for j in range(NJ):
                    art = pfa.tile([128, 512], BF16, tag="art")
                    nc.sync.dma_start(art[:],
                                      ar_out.ap()[g, j * 128:(j + 1) * 128, :])
                    sc = pfa.tile([128, 512], F32, tag="sc")
                    nc.vector.tensor_copy(sc[:], art[:])
                    scat = nc.gpsimd.indirect_dma_start(
                        out=out.ap(),
                        out_offset=IndirectOffsetOnAxis(
                            ap=dest_i[j][:, 0:1], axis=0),
                        in_=sc[:], in_offset=None,
                        element_offset=g * 512,
                        bounds_check=HALF - 1, oob_is_err=False,
                    )
                    for r in residual_dmas:
                        add_dep_helper(scat.ins, r.ins, sync=True,
                                       reason="scatter after residual copy")

    return nc


# ---------------------------------------------------------------------------
# Host-side wrapper
# ---------------------------------------------------------------------------

_BUILT = {}


def _get_nc(S, D, DFF, K):
    key = (S, D, DFF, K)
    if key not in _BUILT:
        from concourse import bacc
        nc = bacc.Bacc(trn_type="TRN2", num_devices=NC_CORES, debug=False)
        build_mod_kernel(nc, S, D, DFF, K)
        nc.compile()
        _BUILT[key] = nc
    return _BUILT[key]


def make_in_maps(x, W_r, b_r, W1, b1, W2, b2, S, D, DFF, K):
    import ml_dtypes
    bf = ml_dtypes.bfloat16
    HALF = S // 2
    DFFH = DFF // 2
    ND = D // 128
    NM = DFFH // 128
    MG, KK, ML = 4, 4, 4
    NMG, NKK, NML = NM // MG, ND // KK, NM // ML
    NGRP = D // 512
    in_maps = []
    w1sh, w2sh, b1sh = [], [], []
    for h in range(2):
        w1s = np.ascontiguousarray(W1[:, h * DFFH:(h + 1) * DFFH]).astype(bf)
        # [d, f] -> [mg, kk, p, kl, i, q]; d = 128*(KK*kk+kl)+p,
        # f = (MG*mg+i)*128+q
        w1sh.append(np.ascontiguousarray(
            w1s.reshape(NKK, KK, 128, NMG, MG, 128)
               .transpose(3, 0, 2, 1, 4, 5)
               .reshape(NMG, NKK, 128, KK * MG * 128)))
        w2s = np.ascontiguousarray(W2[h * DFFH:(h + 1) * DFFH, :]).astype(bf)
        # [dff, d] -> [g, mm, p, ml, q]; dff = 128*(ML*mm+ml)+p, d = 512*g+q
        w2sh.append(np.ascontiguousarray(
            w2s.reshape(NML, ML, 128, NGRP, 512)
               .transpose(3, 0, 2, 1, 4)
               .reshape(NGRP, NML, 128, ML * 512)))
        # b1 pre-transposed to [128, NM]
        b1sh.append(np.ascontiguousarray(
            b1[h * DFFH:(h + 1) * DFFH].reshape(NM, 128).T.astype(np.float32)))
    b2half = (0.5 * np.asarray(b2, dtype=np.float32)).astype(bf).reshape(1, D)
    for c in range(NC_CORES):
        b, h = c // 2, c % 2
        in_maps.append({
            "x_own": np.ascontiguousarray(x[b, h * HALF:(h + 1) * HALF, :]),
            "x_row": np.ascontiguousarray(x[b]),
            "wr": W_r.reshape(1, D).astype(np.float32),
            "br": b_r.reshape(1, 1).astype(np.float32),
            "w1": w1sh[h],
            "w2": w2sh[h],
            "b1s": b1sh[h],
            "b2h": b2half,
            "hoff": np.array([[h * HALF]], dtype=np.float32),
        })
    return in_maps


def kernel(x, W_r, b_r, W1, b1, W2, b2, position_ids=None, cache_position=None,
           **unused):
    x = np.asarray(x, dtype=np.float32)
    W_r = np.asarray(W_r, dtype=np.float32)
    b_r = np.asarray(b_r, dtype=np.float32)
    W1 = np.asarray(W1, dtype=np.float32)
    b1 = np.asarray(b1, dtype=np.float32)
    W2 = np.asarray(W2, dtype=np.float32)
    b2 = np.asarray(b2, dtype=np.float32)
    B, S, D = x.shape
    DFF = W1.shape[1]
    K = 512
    HALF = S // 2
    nc = _get_nc(S, D, DFF, K)
    in_maps = make_in_maps(x, W_r, b_r, W1, b1, W2, b2, S, D, DFF, K)
    res = run_bass_kernel_spmd(nc, in_maps, list(range(NC_CORES)))
    out = np.empty((B, S, D), dtype=np.float32)
    for c in range(NC_CORES):
        b, h = c // 2, c % 2
        out[b, h * HALF:(h + 1) * HALF, :] = res.results[c]["out"]
    return out
